# revision 19
# baseline (speedup 1.0000x reference)
"""TRN2 Bass kernel for gated cross-attention with pair bias (head-sharded, 8 cores).

Reference computation (fp32):
    q = (q_data @ Wq) * kd^-0.5 ; k = m_data @ Wk ; v = m_data @ Wv
    logits = einsum('ihk,jhk->hij', q, k) + pair_bias
    probs  = softmax(logits, -1)
    wa     = einsum('hij,jhk->ihk', probs, v) * sigmoid(q_data @ Wg + bg)
    out    = wa.reshape(AQ, VD) @ Wo + bo

Sharding: 16 heads / 8 cores = 2 heads per core. Each core computes its
head group end-to-end plus a partial output projection (its 128 rows of
Wo); the host sums the 8 partial outputs and adds bo.

On-chip layout is fully transposed (token dim on the free axis):
  S^T[j,i] = khT.T @ qhT                  (PSUM, fp32; q-scale folded
        into Wq on the host)
  E^T = exp(S^T) * exp(pair_bias)^T       (ACT exp from PSUM; pb folded
        in multiplicatively, exp(pb) precomputed on the host)
  [waT ; r] = [v | 1].T @ E^T             (row-sums via ones-columns;
        1/r via GpSimd bcast + fast reciprocal)
  outT = WoS.T @ (waT * gT * (1/r))

v6 pipeline: the two heads are INTERLEAVED per j-tile step.  Their S
matmuls contract over only 64 partitions, so head0 (PE rows 0-63) and
head1 (rows 64-127) execute in separate row-quadrants CONCURRENTLY when
adjacent in the PE queue; emission order h0q0,h1q0,h0q1,h1q1 makes the
4 S matmuls take ~2 matmul slots.  Per step the PE then does 4 S + 4 PV
~= 2.5 matmul-equivalents, which matches the ACT exp cadence even at
the HAM-throttled 1.2GHz clock -- the kernel no longer depends on the
PE clock gate staying warm.  PV trails S by 2 steps so the strict-FIFO
PE queue never waits on the exp->mul chain.  Projection chunks are
drip-fed into early pass-0 steps as ring insertions on the S PSUM tag
(q+k as bank-halves of one tile, v as 4 quarter-groups of a half), and
the output projection runs as a pipelined tail on the same ring.
PSUM: s 2x2banks + pv 4 = 8.  Pair-bias loads ride SWDGE (gpsimd).

All data-side matmuls run in fp16 (inputs are rounded once on the host;
fp16xfp16 products accumulate exactly in fp32 PSUM).
"""

import sys

sys.path.insert(0, "/opt/trn_rl_repo")

import numpy as np

AQ, AM, D, H = 2048, 2048, 1024, 16
KD, VD, OUT = 1024, 1024, 1024
NCORES = 8
HPC = H // NCORES  # heads per core
CW = HPC * (KD // H)  # per-core projection width: 128
DH = KD // H  # head dim: 64

_compiled = None


def _build():
    import concourse.bacc as bacc
    import concourse.mybir as mybir
    import concourse.tile as tile

    f32 = mybir.dt.float32
    bf16 = mybir.dt.float16
    AF = mybir.ActivationFunctionType

    nc = bacc.Bacc(trn_type="TRN2")

    # qdT/mdT: [p, ic, dc, i] flattened so each per-i-chunk DMA reads
    # 8KB contiguous per partition line; pbT likewise [h][p, ps, jt, i];
    # outT is stored [p, ic, oc, i] and unpermuted on the host.
    qdT = nc.declare_dram_parameter("qdT", [128, AQ * D // 128], bf16, isOutput=False)
    mdT = nc.declare_dram_parameter("mdT", [128, AM * D // 128], bf16, isOutput=False)
    pbT = nc.declare_dram_parameter(
        "pbT", [HPC, 128, AM * AQ // 128], bf16, isOutput=False
    )
    wq = nc.declare_dram_parameter("wq", [128, D // 128 * CW], bf16, isOutput=False)
    wk = nc.declare_dram_parameter("wk", [128, D // 128 * CW], bf16, isOutput=False)
    wv = nc.declare_dram_parameter("wv", [128, D // 128 * CW], bf16, isOutput=False)
    wo = nc.declare_dram_parameter("wo", [CW, OUT], bf16, isOutput=False)
    gTx = nc.declare_dram_parameter("gTx", [CW, AQ], bf16, isOutput=False)
    outT = nc.declare_dram_parameter(
        "outT", [128, OUT * AQ // 128], bf16, isOutput=True
    )

    P = 128  # partitions
    NB = 512  # matmul moving-dim block
    NIC = AQ // NB  # 4 i-chunks
    NJT = AM // P  # 16 j-tiles
    NDC = D // P  # 8 contraction chunks
    NBP = 2 * NB  # 1024 columns per pass
    NPS = 2  # passes

    with tile.TileContext(nc) as tc:
        with (
            tc.tile_pool(name="consts", bufs=1) as consts,
            tc.tile_pool(name="proj", bufs=1) as proj,
            tc.tile_pool(name="stream", bufs=2) as stream,
            tc.tile_pool(name="pbpool", bufs=3) as pbpool,
            tc.tile_pool(name="tsbp", bufs=2) as tsbp,
            tc.tile_pool(name="etp", bufs=5) as etp,
            tc.tile_pool(name="fin", bufs=1) as fin,
            tc.tile_pool(name="osbp", bufs=2) as osbp,
            tc.tile_pool(name="wagp", bufs=1) as wagp,
        ):
            # ---- constants + input streams, ordered by first use ----
            wq_sb = consts.tile([P, NDC, CW], bf16, tag="wq_sb")
            wk_sb = consts.tile([P, NDC, CW], bf16, tag="wk_sb")
            wv_sb = consts.tile([P, NDC, CW], bf16, tag="wv_sb")
            nc.sync.dma_start(wq_sb[:], wq.rearrange("p (dc c) -> p dc c", dc=NDC))

            qdmd = {}

            def emit_proj_load(ic):
                qd = stream.tile([P, NDC, NB], bf16, tag="qd", name=f"qd_{ic}")
                md = stream.tile([P, NDC, NB], bf16, tag="md", name=f"md_{ic}")
                csz = NDC * NB
                nc.sync.dma_start(
                    qd[:],
                    qdT[:, ic * csz : (ic + 1) * csz].rearrange(
                        "p (dc i) -> p dc i", dc=NDC
                    ),
                )
                nc.sync.dma_start(
                    md[:],
                    mdT[:, ic * csz : (ic + 1) * csz].rearrange(
                        "p (dc i) -> p dc i", dc=NDC
                    ),
                )
                qdmd[ic] = (qd, md)

            emit_proj_load(0)
            nc.sync.dma_start(wk_sb[:], wk.rearrange("p (dc c) -> p dc c", dc=NDC))
            nc.sync.dma_start(wv_sb[:], wv.rearrange("p (dc c) -> p dc c", dc=NDC))

            # preload the exp table set (avoids the ~2.7us first-call
            # ACT_TABLE_LOAD) and a scratch tile for PE warm-up matmuls
            warm = consts.tile([1, 8], f32, tag="warm")
            nc.vector.memset(warm[:], 0.0)
            warm2 = consts.tile([1, 8], f32, tag="warm2")
            nc.scalar.activation(warm2[:], warm[:], AF.Exp)
            scr = consts.tile([P, P], bf16, tag="scr")
            nc.vector.memset(scr[:], 0.0)

            def load_pb(ps, h):
                pb = pbpool.tile([P, NJT, NBP], bf16, tag="pb", name=f"pb_{ps}_{h}")
                psz = NJT * NBP
                nc.gpsimd.dma_start(
                    pb[:],
                    pbT[h, :, ps * psz : (ps + 1) * psz].rearrange(
                        "p (jt i) -> p jt i", jt=NJT
                    ),
                )
                return pb

            pb_tiles = {(0, 0): load_pb(0, 0), (0, 1): load_pb(0, 1)}

            emit_proj_load(1)
            wo_sb = consts.tile([P, OUT], bf16, tag="wo_sb")
            nc.sync.dma_start(wo_sb[:], wo[:])
            gT = consts.tile([P, AQ], bf16, tag="gT")
            nc.sync.dma_start(gT[:], gTx[:])

            # ---- projection targets ----
            qhT = proj.tile([P, AQ], bf16, tag="qhT")
            khT = proj.tile([P, AM], bf16, tag="khT")
            v1 = [
                proj.tile([P, 2 * P], bf16, tag=f"v1_{j}", name=f"v1_{j}")
                for j in range(NJT)
            ]
            for jt in range(NJT):
                nc.vector.memset(v1[jt][:, DH:P], 1.0)
                nc.vector.memset(v1[jt][:, P + DH : 2 * P], 1.0)

            # ---- PSUM: s-ring (2 x [128,1024] = 4 banks) + pv (4 banks).
            # Projection drips, warm-up spam, and the outproj tail all
            # allocate "sps"-tagged ring tiles instead of extra banks.
            s_ctx = tc.tile_pool(name="s_ps", bufs=2, space="PSUM")
            s_ps = s_ctx.__enter__()
            pv_ctx = tc.tile_pool(name="pv_ps", bufs=4, space="PSUM")
            pv_ps = pv_ctx.__enter__()

            nring = [0]

            def ring_tile(label):
                nring[0] += 1
                return s_ps.tile(
                    [P, NBP], f32, tag="sps", name=f"{label}_{nring[0]}"
                )

            # PE warm-up spam: ~10 junk matmuls so the HAM clock gate is
            # released before the real projection work arrives.
            wu = ring_tile("warmup")
            for k in range(24):
                nc.tensor.matmul(
                    wu[:, 0:P], scr[:], scr[:], start=k == 0, stop=k == 23
                )

            def emit_qk(ic):
                # q and k projections of chunk ic into the two bank-
                # halves of one ring tile (parallel accumulation chains)
                t = ring_tile(f"pqk{ic}")
                qd, md = qdmd[ic]
                for dc in range(NDC):
                    st, sp = dc == 0, dc == NDC - 1
                    nc.tensor.matmul(
                        t[:, 0:NB], wq_sb[:, dc, :], qd[:, dc, :], start=st, stop=sp
                    )
                    nc.tensor.matmul(
                        t[:, NB:NBP], wk_sb[:, dc, :], md[:, dc, :], start=st, stop=sp
                    )
                sl = slice(ic * NB, (ic + 1) * NB)
                nc.scalar.copy(qhT[:, sl], t[:, 0:NB])
                nc.vector.tensor_copy(khT[:, sl], t[:, NB:NBP])

            def emit_v(ic):
                t = ring_tile(f"pv{ic}")
                md = qdmd[ic][1]
                for dc in range(NDC):
                    for t4 in range(NB // P):
                        # only the first matmul into the bank may set
                        # start (it clears has_written bank-wide)
                        nc.tensor.matmul(
                            t[:, t4 * P : (t4 + 1) * P],
                            md[:, dc, t4 * P : (t4 + 1) * P],
                            wv_sb[:, dc, :],
                            start=(dc == 0 and t4 == 0),
                            stop=(dc == NDC - 1 and t4 == NB // P - 1),
                            skip_group_check=True,
                        )
                for t4 in range(NB // P):
                    jt = ic * (NB // P) + t4
                    nc.scalar.copy(v1[jt][:, 0:DH], t[:, t4 * P : t4 * P + DH])
                    nc.scalar.copy(
                        v1[jt][:, P : P + DH],
                        t[:, t4 * P + DH : t4 * P + 2 * DH],
                    )

            # up-front: only q0+k0 and q1 (k for j-tiles 0-3 and q for
            # pass-0 columns); everything else drips into pass 0.
            emit_qk(0)
            emit_proj_load(2)
            emit_qk(1)
            emit_proj_load(3)
            deferred = {
                0: lambda: emit_v(0),
                1: lambda: emit_v(1),
                3: lambda: emit_qk(2),
                5: lambda: emit_v(2),
                7: lambda: emit_qk(3),
                9: lambda: emit_v(3),
            }

            # ---- attention: heads interleaved per j-tile step ----
            wag = [
                wagp.tile([P, NB], bf16, tag=f"wag{i}", name=f"wag_{i}")
                for i in range(NIC)
            ]

            for ps in range(NPS):
                pbs = {h: pb_tiles.pop((ps, h)) for h in range(HPC) if (ps, h) in pb_tiles}
                pvs = {
                    h: [
                        pv_ps.tile(
                            [P, NB], f32, tag="pvs", name=f"pvs_{ps}_{h}_{q}"
                        )
                        for q in range(2)
                    ]
                    for h in range(HPC)
                }
                ets = {}

                def emit_pv(h, jt):
                    et = ets.pop((h, jt))
                    vcol = slice(h * P, (h + 1) * P)
                    for q in range(2):
                        nc.tensor.matmul(
                            pvs[h][q][:],
                            v1[jt][:, vcol],
                            et[:, q * NB : (q + 1) * NB],
                            start=(jt == 0),
                            stop=(jt == NJT - 1),
                        )

                for jt in range(NJT):
                    sps = {
                        h: s_ps.tile(
                            [P, NBP], f32, tag="sps", name=f"sps_{ps}_{jt}_{h}"
                        )
                        for h in range(2)
                    }
                    # h0/h1 S matmuls adjacent: separate row-quadrants,
                    # execute concurrently
                    for q in range(2):
                        for h in range(2):
                            hs = slice(h * DH, (h + 1) * DH)
                            nc.tensor.matmul(
                                sps[h][:, q * NB : (q + 1) * NB],
                                khT[hs, jt * P : (jt + 1) * P],
                                qhT[hs, (ps * 2 + q) * NB : (ps * 2 + q + 1) * NB],
                                start=True,
                                stop=True,
                            )
                    for h in range(2):
                        tsb = tsbp.tile([P, NBP], bf16, tag="tsb")
                        et = etp.tile([P, NBP], bf16, tag="et")
                        nc.scalar.activation(tsb[:], sps[h][:], AF.Exp)
                        nc.vector.tensor_mul(et[:], tsb[:], pbs[h][:, jt, :])
                        ets[(h, jt)] = et
                    if jt >= 2:
                        emit_pv(0, jt - 2)
                        emit_pv(1, jt - 2)
                    if ps == 0:
                        act = deferred.pop(jt, None)
                        if act is not None:
                            act()
                        if jt == 11:
                            pb_tiles[(1, 0)] = load_pb(1, 0)
                        if jt == NJT - 1:
                            # slot 0 (pb 0,0) was last read by the h0 mul
                            # of this step, emitted just above
                            pb_tiles[(1, 1)] = load_pb(1, 1)
                for h in range(2):
                    emit_pv(h, NJT - 2)
                    emit_pv(h, NJT - 1)

                # finalize both heads: wa * gate / rowsum
                for h in range(2):
                    hs = slice(h * DH, (h + 1) * DH)
                    rec = fin.tile([1, NBP], f32, tag="rec")
                    tg = fin.tile([DH, NBP], f32, tag="tg")
                    for q in range(2):
                        ic = ps * 2 + q
                        qsl = slice(q * NB, (q + 1) * NB)
                        nc.vector.tensor_copy(
                            rec[:, qsl], pvs[h][q][DH : DH + 1, :]
                        )
                        nc.vector.tensor_mul(
                            tg[:, qsl],
                            pvs[h][q][0:DH, :],
                            gT[hs, ic * NB : (ic + 1) * NB],
                        )
                    rb = fin.tile([DH, NBP], f32, tag="rb")
                    nc.gpsimd.partition_broadcast(rb[:], rec[0:1, :])
                    rbc = fin.tile([DH, NBP], f32, tag="rbc")
                    nc.vector.reciprocal_approx_fast(rbc[:], rb[:])
                    for q in range(2):
                        ic = ps * 2 + q
                        qsl = slice(q * NB, (q + 1) * NB)
                        nc.vector.tensor_mul(
                            wag[ic][hs, :], tg[:, qsl], rbc[:, qsl]
                        )

            # ---- output projection tail: pairs of (ic,oc) units per
            # ring tile (one bank-half each), evacuation alternating
            # between DVE and ACT, one 1MB store per i-chunk ----
            osb_big = {}
            units = [(ic, oc) for ic in range(NIC) for oc in range(OUT // P)]
            for k in range(0, len(units), 2):
                t = ring_tile(f"po{k}")
                for j, (ic, oc) in enumerate(units[k : k + 2]):
                    half = slice(j * NB, (j + 1) * NB)
                    nc.tensor.matmul(
                        t[:, half],
                        wo_sb[:, oc * P : (oc + 1) * P],
                        wag[ic][:],
                        start=True,
                        stop=True,
                    )
                for j, (ic, oc) in enumerate(units[k : k + 2]):
                    half = slice(j * NB, (j + 1) * NB)
                    if oc == 0:
                        osb_big[ic] = osbp.tile(
                            [P, OUT // P, NB], bf16, tag="osb", name=f"osb_{ic}"
                        )
                    if oc % 2 == 0:
                        nc.vector.tensor_copy(osb_big[ic][:, oc, :], t[:, half])
                    else:
                        nc.scalar.copy(osb_big[ic][:, oc, :], t[:, half])
                    if oc == OUT // P - 1:
                        osz = (OUT // P) * NB
                        nc.sync.dma_start(
                            outT[:, ic * osz : (ic + 1) * osz].rearrange(
                                "p (oc i) -> p oc i", oc=OUT // P
                            ),
                            osb_big[ic][:],
                        )

            pv_ctx.__exit__(None, None, None)
            s_ctx.__exit__(None, None, None)

    nc.compile()
    return nc


def _get_compiled():
    global _compiled
    if _compiled is None:
        _compiled = _build()
    return _compiled


def _sigmoid(x):
    return 1.0 / (1.0 + np.exp(-x))


def _wperm(w):
    """[D, CW] -> [128, (D//128)*CW]: per-partition-contiguous weight layout."""
    d, cw = w.shape
    return np.ascontiguousarray(
        w.reshape(d // 128, 128, cw).transpose(1, 0, 2).reshape(128, -1)
    )


def kernel(q_data, m_data, bias, pair_bias, Wq, Wk, Wv, Wg, bg, Wo, bo):
    from concourse.bass_utils import run_bass_kernel_spmd

    q_data = np.asarray(q_data, dtype=np.float32)
    m_data = np.asarray(m_data, dtype=np.float32)
    pair_bias = np.asarray(pair_bias, dtype=np.float32)
    Wq = np.asarray(Wq, dtype=np.float32)
    Wk = np.asarray(Wk, dtype=np.float32)
    Wv = np.asarray(Wv, dtype=np.float32)
    Wg = np.asarray(Wg, dtype=np.float32)
    bg = np.asarray(bg, dtype=np.float32)
    Wo = np.asarray(Wo, dtype=np.float32)
    bo = np.asarray(bo, dtype=np.float32)

    nc = _get_compiled()

    bf = np.float16
    NDC, NIC, NB, NJT, NPS, NBP = D // 128, AQ // 512, 512, AM // 128, 2, 1024

    def _dma_layout(x):
        # [tok, d] -> [p, ic, dc, i] flattened: per-i-chunk contiguous
        return np.ascontiguousarray(
            x.T.reshape(NDC, 128, NIC, NB).transpose(1, 2, 0, 3).reshape(128, -1)
        ).astype(bf)

    qdT = _dma_layout(q_data)
    mdT = _dma_layout(m_data)
    SCALE = float(DH) ** -0.5
    Wq_s = Wq * SCALE  # fold the q scale into the weights

    in_maps = []
    for c in range(NCORES):
        cs = slice(c * CW, (c + 1) * CW)
        in_maps.append(
            {
                "qdT": qdT,
                "mdT": mdT,
                "pbT": np.ascontiguousarray(
                    np.exp(pair_bias[c * HPC : (c + 1) * HPC].transpose(0, 2, 1))
                    .reshape(HPC, NJT, 128, NPS, NBP)
                    .transpose(0, 2, 3, 1, 4)
                    .reshape(HPC, 128, -1)
                ).astype(bf),
                "wq": _wperm(Wq_s[:, cs]).astype(bf),
                "wk": _wperm(Wk[:, cs]).astype(bf),
                "wv": _wperm(Wv[:, cs]).astype(bf),
                "wo": np.ascontiguousarray(Wo[cs, :]).astype(bf),
                "gTx": np.ascontiguousarray(
                    _sigmoid(q_data @ Wg[:, cs] + bg[cs]).T
                ).astype(bf),
            }
        )

    global _last_in_maps
    _last_in_maps = in_maps
    res = run_bass_kernel_spmd(nc, in_maps, core_ids=list(range(NCORES)))
    out = np.zeros((AQ, OUT), dtype=np.float32)
    for c in range(NCORES):
        # [p, ic, oc, i] -> [tok, out]
        o = res.results[c]["outT"].reshape(128, NIC, OUT // 128, NB)
        out += o.transpose(2, 0, 1, 3).reshape(OUT, AQ).T.astype(np.float32)
    out += bo
    return out
